# revision 14
# baseline (speedup 1.0000x reference)
"""BranchLinear (MoE routing) Trainium2 kernel.

Math: out[t] = x[t] @ weight[branch_idx[t]] + bias[branch_idx[t]]
  x: [131072, 512] f32, branch_idx: [131072] int32 in [0,8),
  weight: [8, 512, 512] f32, bias: [8, 512] f32.

Strategy (data-parallel over 8 NeuronCores, T sharded):
  Per core (16384 tokens): tokens are processed grouped by branch so each
  token is multiplied by exactly one 512x512 weight (1x FLOPs, vs 8x for
  the masked approach). The grouping permutation is cheap host-side
  bookkeeping (argsort of the given routing); all FLOPs and all HBM
  traffic (x gather, weight load, out scatter) happen on device:
    - one batched dma_gather (mlp GPSIMD library) fetches CH*128 sorted
      token rows per instruction into SBUF; slot i lands at partition
      i%128, tile i//128 — exactly the layout the compute consumes. The
      batching amortizes the ~1us SWDGE fixed cost over CH tiles.
    - PE transposes each 128-token tile (via identity matmul) so D is on
      partitions; all matmul data is typed float32r, which streams at
      1 cycle/row on the PE (plain fp32 runs at 1/4 rate)
    - 4 accumulated matmuls against the resident branch weight
    - DVE adds the (pre-broadcast) branch bias
    - a per-tile indirect DMA scatters 128 result rows to their original
      slots (the HW consumes exactly one scatter index per SBUF
      partition, so scatters cannot be batched the same way)
  Branch segments are padded to 128-token tiles (pad slots gather row 0
  and scatter into trash rows TS..TS+127), and per-branch slot sizes are
  the max over cores so one SPMD program serves all 8 cores.
"""

import numpy as np

P = 128           # SBUF partitions / tile height (tokens per tile)
CH = 8            # 128-token tiles per dma_gather
NCORES = 8

_prog_cache = {}


def _split_multiwaits(nc):
    """This container's walrus build allows at most ONE sync wait per
    instruction (2 for EventSemaphore), but Tile emits instructions with
    several waits. Hoist extra waits onto fresh single-wait nops inserted
    just before the instruction on the same engine (identical blocking
    semantics: the engine's sequencer executes both in program order)."""
    import concourse.mybir as mybir

    uid = 0
    for f in nc.m.functions:
        for bb in f.blocks:
            insts = bb.instructions
            out, changed = [], False
            for ins in insts:
                si = ins.sync_info
                cap = 2 if ins.opcode == "EventSemaphore" else 1
                if si is not None and len(si.on_wait) > cap:
                    waits = list(si.on_wait)
                    for w in waits[cap:]:
                        nop = mybir.InstNoOp(
                            name=f"waitsplit_{uid}",
                            engine=ins.engine,
                            bass_nofuse=True,
                            sync_info=mybir.SyncInfo(on_wait=[w], on_update=[]),
                        )
                        uid += 1
                        nc.register_instruction(nop, overwrite=True)
                        out.append(nop)
                    si.on_wait = waits[:cap]
                    ins.sync_info = si
                    changed = True
                out.append(ins)
            if changed:
                bb.instructions = out
    return nc


def _build_program(TS, D, NB, branch_of_tile, epochs=1):
    """Build the per-core SPMD bass program.

    Inputs (per core): x [TS, D] f32r, wr [NB*D, D] f32r (weight reshaped),
    br [1, NB*D] f32, gidx16 [P, S*8] int16 (dma_gather index layout:
    slot i at partition i%16, column i//16, replicated over the 8
    16-partition groups; pads read row 0), sidx [P, S] int32 (scatter
    pads write trash rows TS..TS+127). Output: out [TS+P, D] f32 (last P
    rows are trash). f32r is bit-identical to f32; it only switches the
    PE streaming mode.
    """
    import concourse.bass as bass
    import concourse.mybir as mybir
    import concourse.tile as tile
    from concourse import library_config
    from concourse.library_overlay import lower_extended_insts
    from concourse.masks import make_identity

    f32 = mybir.dt.float32
    f32r = mybir.dt.float32r
    KC = D // P                       # contraction chunks (4)
    S = len(branch_of_tile)           # total 128-token tiles
    assert S % CH == 0

    nc = bass.Bass(name="branch_linear")
    x_d = nc.dram_tensor("x", [TS, D], f32r, kind="ExternalInput")
    w_d = nc.dram_tensor("wr", [NB * D, D], f32r, kind="ExternalInput")
    b_d = nc.dram_tensor("br", [1, NB * D], f32, kind="ExternalInput")
    gidx_d = nc.dram_tensor("gidx16", [P, S * 8], mybir.dt.int16,
                            kind="ExternalInput")
    sidx_d = nc.dram_tensor("sidx", [P, S], mybir.dt.int32, kind="ExternalInput")
    out_d = nc.dram_tensor("out", [TS + P, D], f32, kind="ExternalOutput")

    with tile.TileContext(nc) as tc:
        with (
            tc.tile_pool(name="const", bufs=1) as cpool,
            tc.tile_pool(name="gather", bufs=2) as gpool,
            tc.tile_pool(name="xt", bufs=4) as tpool,
            tc.tile_pool(name="osb", bufs=4) as opool,
            tc.tile_pool(name="ps_t", bufs=2, space="PSUM") as ps_t,
            tc.tile_pool(name="ps_o", bufs=2, space="PSUM") as ps_o,
            tc.tile_pool(name="ps_b", bufs=1, space="PSUM") as ps_b,
        ):
            # memset rejects f32r, so build the identity in f32 and convert
            # with a (bit-identical) copy
            ident_f32 = cpool.tile([P, P], f32, tag="ident_f32")
            make_identity(nc, ident_f32[:])
            ident = cpool.tile([P, P], f32r, tag="ident")
            nc.vector.tensor_copy(out=ident[:], in_=ident_f32[:])

            # dma_gather lives in the mlp GPSIMD library; load it after the
            # make_identity gpsimd ops (Pool program order is preserved)
            nc.gpsimd.load_library(library_config.mlp)
            # one shared constant register: to_reg inside every dma_gather
            # call would exhaust the Pool register file on epochs>1 builds
            nidx_reg = nc.gpsimd.to_reg(CH * P)

            gidx_sb = cpool.tile([P, S * 8], mybir.dt.int16, tag="gidx16")
            nc.sync.dma_start(gidx_sb[:], gidx_d[:, :])
            sidx_sb = cpool.tile([P, S], mybir.dt.int32, tag="sidx")
            nc.sync.dma_start(sidx_sb[:], sidx_d[:, :])

            # resident weights: one [P, D] SBUF tile per (branch, k-chunk)
            w_sb = {}
            for n in range(NB):
                for k in range(KC):
                    w = cpool.tile([P, D], f32r, tag=f"w_{n}_{k}")
                    r0 = (n * KC + k) * P
                    nc.sync.dma_start(w[:], w_d[r0:r0 + P, :])
                    w_sb[(n, k)] = w

            # bias, broadcast to 128 partitions via K=1 matmul with ones
            bias1p = cpool.tile([1, NB * D], f32, tag="bias1p")
            nc.sync.dma_start(bias1p[:], b_d[:, :])
            ones1p = cpool.tile([1, P], f32, tag="ones1p")
            nc.vector.memset(ones1p[:], 1.0)
            bias_bc = cpool.tile([P, NB * D], f32, tag="bias_bc")
            for n in range(NB):
                pb = ps_b.tile([P, D], f32)
                nc.tensor.matmul(
                    out=pb[:], lhsT=ones1p[:], rhs=bias1p[:, n * D:(n + 1) * D],
                    start=True, stop=True,
                )
                nc.scalar.copy(out=bias_bc[:, n * D:(n + 1) * D], in_=pb[:])

            for s0 in list(range(0, S, CH)) * epochs:
                # 1) batched gather: CH*128 sorted token rows; slot i lands
                #    at partition i%128, tile i//128 (pads read row 0)
                xg = gpool.tile([P, CH, D], f32r, tag="xg")
                nc.gpsimd.dma_gather(
                    xg[:], x_d[:, :],
                    gidx_sb[:, s0 * 8:(s0 + CH) * 8],
                    CH * P, nidx_reg, D,
                )
                for g in range(CH):
                    n = branch_of_tile[s0 + g]
                    # 2) transpose tile so D is on partitions (PE, identity)
                    xt_ps = ps_t.tile([P, D], f32r)
                    for k in range(KC):
                        nc.tensor.transpose(
                            out=xt_ps[:, k * P:(k + 1) * P],
                            in_=xg[:, g, k * P:(k + 1) * P],
                            identity=ident[:],
                        )
                    xt = tpool.tile([P, D], f32r, tag="xt")
                    nc.scalar.copy(out=xt[:], in_=xt_ps[:])
                    # 3) out[tok, :] = sum_k xt[:,k].T @ W[n][k]
                    o_ps = ps_o.tile([P, D], f32)
                    for k in range(KC):
                        nc.tensor.matmul(
                            out=o_ps[:],
                            lhsT=xt[:, k * P:(k + 1) * P],
                            rhs=w_sb[(n, k)][:],
                            start=(k == 0), stop=(k == KC - 1),
                        )
                    # 4) + bias (PSUM -> SBUF)
                    o_sb = opool.tile([P, D], f32, tag="osb")
                    nc.vector.tensor_add(
                        out=o_sb[:], in0=o_ps[:],
                        in1=bias_bc[:, n * D:(n + 1) * D],
                    )
                    # 5) scatter rows to original slots (pads -> trash rows)
                    nc.gpsimd.indirect_dma_start(
                        out=out_d[:, :],
                        out_offset=bass.IndirectOffsetOnAxis(
                            ap=sidx_sb[:, s0 + g:s0 + g + 1], axis=0),
                        in_=o_sb[:], in_offset=None,
                    )
    # Raw Bass skips Bacc's codegen_inst_isa_subclasses pass; without it the
    # library-reload InstISA has empty instr bytes ("ISA wrong length")
    lower_extended_insts(nc)
    return _split_multiwaits(nc)


def _routing(branch_idx, TS, NB):
    """Per-core padded, branch-sorted gather/scatter index arrays.

    Returns (gidx16 [NCORES][P, S*8] int16, sidx [NCORES][P, S] int32,
    branch_of_tile [S]). Gather pads read row 0; scatter pads write trash
    rows TS + partition."""
    ncores = branch_idx.shape[0] // TS
    perms, counts = [], np.zeros((ncores, NB), np.int64)
    for c in range(ncores):
        bi = branch_idx[c * TS:(c + 1) * TS]
        perms.append(np.argsort(bi, kind="stable"))
        counts[c] = np.bincount(bi, minlength=NB)
    slot_tiles = [int(-(-counts[:, n].max() // P)) for n in range(NB)]
    branch_of_tile = []
    for n in range(NB):
        branch_of_tile += [n] * slot_tiles[n]
    while len(branch_of_tile) % CH:      # pad to whole gather chunks
        branch_of_tile.append(NB - 1)
    S = len(branch_of_tile)

    gidx_arrays, sidx_arrays = [], []
    for c in range(ncores):
        flat = np.full(S * P, -1, np.int64)
        off = base = 0
        for n in range(NB):
            cnt = int(counts[c, n])
            flat[base:base + cnt] = perms[c][off:off + cnt]
            off += cnt
            base += slot_tiles[n] * P
        pad = flat < 0
        gflat = np.where(pad, 0, flat)
        sflat = np.where(pad, TS + (np.arange(S * P) % P), flat)
        # dma_gather index layout: slot i -> partition i%16, column i//16,
        # replicated across the 8 16-partition groups
        g16 = np.tile(gflat.reshape(-1, 16).T.astype(np.int16), (8, 1))
        gidx_arrays.append(np.ascontiguousarray(g16))
        sidx_arrays.append(
            np.ascontiguousarray(sflat.reshape(S, P).T.astype(np.int32)))
    return gidx_arrays, sidx_arrays, branch_of_tile


def kernel(x, branch_idx, weight, bias):
    from concourse.bass_utils import run_bass_kernel_spmd

    x = np.ascontiguousarray(np.asarray(x, np.float32))
    branch_idx = np.asarray(branch_idx, np.int32)
    weight = np.ascontiguousarray(np.asarray(weight, np.float32))
    bias = np.ascontiguousarray(np.asarray(bias, np.float32))

    T, D = x.shape
    NB = weight.shape[0]
    TS = T // NCORES

    gidx_arrays, sidx_arrays, branch_of_tile = _routing(branch_idx, TS, NB)

    key = (TS, D, NB, tuple(branch_of_tile))
    if key not in _prog_cache:
        _prog_cache[key] = _build_program(TS, D, NB, branch_of_tile)
    nc = _prog_cache[key]

    wr = np.ascontiguousarray(weight.reshape(NB * D, D))
    br = np.ascontiguousarray(bias.reshape(1, NB * D))
    in_maps = [
        {"x": x[c * TS:(c + 1) * TS], "wr": wr, "br": br,
         "gidx16": gidx_arrays[c], "sidx": sidx_arrays[c]}
        for c in range(NCORES)
    ]
    res = run_bass_kernel_spmd(nc, in_maps, core_ids=list(range(NCORES)))
    out = np.concatenate(
        [res.results[c]["out"][:TS] for c in range(NCORES)], axis=0)
    return out


# revision 15
# speedup vs baseline: 1.1850x; 1.1850x over previous
"""BranchLinear (MoE routing) Trainium2 kernel.

Math: out[t] = x[t] @ weight[branch_idx[t]] + bias[branch_idx[t]]
  x: [131072, 512] f32, branch_idx: [131072] int32 in [0,8),
  weight: [8, 512, 512] f32, bias: [8, 512] f32.

Strategy (data-parallel over 8 NeuronCores, T sharded):
  Per core (16384 tokens): tokens are processed grouped by branch so each
  token is multiplied by exactly one 512x512 weight (1x FLOPs, vs 8x for
  the masked approach). The grouping permutation is cheap host-side
  bookkeeping (argsort of the given routing); all FLOPs and all HBM
  traffic (x gather, weight load, out scatter) happen on device:
    - one batched dma_gather (mlp GPSIMD library) fetches CH*128 sorted
      token rows per instruction into SBUF; slot i lands at partition
      i%128, tile i//128 — exactly the layout the compute consumes. The
      batching amortizes the ~1us SWDGE fixed cost over CH tiles.
    - PE transposes each 128-token tile (via identity matmul) so D is on
      partitions; all matmul data is typed float32r, which streams at
      1 cycle/row on the PE (plain fp32 runs at 1/4 rate)
    - 4 accumulated matmuls against the resident branch weight
    - DVE adds the (pre-broadcast) branch bias
    - a per-tile indirect DMA scatters 128 result rows to their original
      slots (the HW consumes exactly one scatter index per SBUF
      partition, so scatters cannot be batched the same way)
  Branch segments are padded to 128-token tiles (pad slots gather row 0
  and scatter into trash rows TS..TS+127), and per-branch slot sizes are
  the max over cores so one SPMD program serves all 8 cores.
"""

import numpy as np

P = 128           # SBUF partitions / tile height (tokens per tile)
CH = 8            # 128-token tiles per dma_gather
NCORES = 8

_prog_cache = {}


def _split_multiwaits(nc):
    """This container's walrus build allows at most ONE sync wait per
    instruction (2 for EventSemaphore), but Tile emits instructions with
    several waits. Hoist extra waits onto fresh single-wait nops inserted
    just before the instruction on the same engine (identical blocking
    semantics: the engine's sequencer executes both in program order)."""
    import concourse.mybir as mybir

    uid = 0
    for f in nc.m.functions:
        for bb in f.blocks:
            insts = bb.instructions
            out, changed = [], False
            for ins in insts:
                si = ins.sync_info
                cap = 2 if ins.opcode == "EventSemaphore" else 1
                if si is not None and len(si.on_wait) > cap:
                    waits = list(si.on_wait)
                    for w in waits[cap:]:
                        nop = mybir.InstNoOp(
                            name=f"waitsplit_{uid}",
                            engine=ins.engine,
                            bass_nofuse=True,
                            sync_info=mybir.SyncInfo(on_wait=[w], on_update=[]),
                        )
                        uid += 1
                        nc.register_instruction(nop, overwrite=True)
                        out.append(nop)
                    si.on_wait = waits[:cap]
                    ins.sync_info = si
                    changed = True
                out.append(ins)
            if changed:
                bb.instructions = out
    return nc


def _build_program(TS, D, NB, branch_of_tile, epochs=1):
    """Build the per-core SPMD bass program.

    Inputs (per core): x [TS, D] f32r, wr [NB*D, D] f32r (weight reshaped),
    br [1, NB*D] f32, gidx16 [P, S*8] int16 (dma_gather index layout:
    slot i at partition i%16, column i//16, replicated over the 8
    16-partition groups; pads read row 0), sidx [P, S] int32 (scatter
    pads write trash rows TS..TS+127). Output: out [TS+P, D] f32 (last P
    rows are trash). f32r is bit-identical to f32; it only switches the
    PE streaming mode.
    """
    import concourse.bass as bass
    import concourse.mybir as mybir
    import concourse.tile as tile
    from concourse import library_config
    from concourse.library_overlay import lower_extended_insts
    from concourse.masks import make_identity

    f32 = mybir.dt.float32
    f32r = mybir.dt.float32r
    KC = D // P                       # contraction chunks (4)
    S = len(branch_of_tile)           # total 128-token tiles
    assert S % CH == 0

    nc = bass.Bass(name="branch_linear")
    x_d = nc.dram_tensor("x", [TS, D], f32r, kind="ExternalInput")
    w_d = nc.dram_tensor("wr", [NB * D, D], f32r, kind="ExternalInput")
    b_d = nc.dram_tensor("br", [1, NB * D], f32, kind="ExternalInput")
    gidx_d = nc.dram_tensor("gidx16", [P, S * 8], mybir.dt.int16,
                            kind="ExternalInput")
    sidx_d = nc.dram_tensor("sidx", [P, S], mybir.dt.int32, kind="ExternalInput")
    out_d = nc.dram_tensor("out", [TS + P, D], f32, kind="ExternalOutput")

    with tile.TileContext(nc) as tc:
        with (
            tc.tile_pool(name="const", bufs=1) as cpool,
            tc.tile_pool(name="gather", bufs=2) as gpool,
            tc.tile_pool(name="xt", bufs=4) as tpool,
            tc.tile_pool(name="osb", bufs=4) as opool,
            tc.tile_pool(name="ps_t", bufs=2, space="PSUM") as ps_t,
            tc.tile_pool(name="ps_o", bufs=2, space="PSUM") as ps_o,
            tc.tile_pool(name="ps_b", bufs=1, space="PSUM") as ps_b,
        ):
            # memset rejects f32r, so build the identity in f32 and convert
            # with a (bit-identical) copy
            ident_f32 = cpool.tile([P, P], f32, tag="ident_f32")
            make_identity(nc, ident_f32[:])
            ident = cpool.tile([P, P], f32r, tag="ident")
            nc.vector.tensor_copy(out=ident[:], in_=ident_f32[:])

            # dma_gather lives in the mlp GPSIMD library; load it after the
            # make_identity gpsimd ops (Pool program order is preserved)
            nc.gpsimd.load_library(library_config.mlp)
            # one shared constant register: to_reg inside every dma_gather
            # call would exhaust the Pool register file on epochs>1 builds
            nidx_reg = nc.gpsimd.to_reg(CH * P)

            gidx_sb = cpool.tile([P, S * 8], mybir.dt.int16, tag="gidx16")
            nc.sync.dma_start(gidx_sb[:], gidx_d[:, :])
            sidx_sb = cpool.tile([P, S], mybir.dt.int32, tag="sidx")
            nc.sync.dma_start(sidx_sb[:], sidx_d[:, :])

            # resident weights: one [P, D] SBUF tile per (branch, k-chunk)
            w_sb = {}
            for n in range(NB):
                for k in range(KC):
                    w = cpool.tile([P, D], f32r, tag=f"w_{n}_{k}")
                    r0 = (n * KC + k) * P
                    nc.sync.dma_start(w[:], w_d[r0:r0 + P, :])
                    w_sb[(n, k)] = w

            # bias, broadcast to 128 partitions via K=1 matmul with ones
            bias1p = cpool.tile([1, NB * D], f32, tag="bias1p")
            nc.sync.dma_start(bias1p[:], b_d[:, :])
            ones1p = cpool.tile([1, P], f32, tag="ones1p")
            nc.vector.memset(ones1p[:], 1.0)
            bias_bc = cpool.tile([P, NB * D], f32, tag="bias_bc")
            for n in range(NB):
                pb = ps_b.tile([P, D], f32)
                nc.tensor.matmul(
                    out=pb[:], lhsT=ones1p[:], rhs=bias1p[:, n * D:(n + 1) * D],
                    start=True, stop=True,
                )
                nc.scalar.copy(out=bias_bc[:, n * D:(n + 1) * D], in_=pb[:])

            for s0 in list(range(0, S, CH)) * epochs:
                # 1) batched gather: CH*128 sorted token rows; slot i lands
                #    at partition i%128, tile i//128 (pads read row 0)
                xg = gpool.tile([P, CH, D], f32r, tag="xg")
                nc.gpsimd.dma_gather(
                    xg[:], x_d[:, :],
                    gidx_sb[:, s0 * 8:(s0 + CH) * 8],
                    CH * P, nidx_reg, D,
                    single_packet=False,
                )
                for g in range(CH):
                    n = branch_of_tile[s0 + g]
                    # 2) transpose tile so D is on partitions (PE, identity)
                    xt_ps = ps_t.tile([P, D], f32r)
                    for k in range(KC):
                        nc.tensor.transpose(
                            out=xt_ps[:, k * P:(k + 1) * P],
                            in_=xg[:, g, k * P:(k + 1) * P],
                            identity=ident[:],
                        )
                    xt = tpool.tile([P, D], f32r, tag="xt")
                    nc.scalar.copy(out=xt[:], in_=xt_ps[:])
                    # 3) out[tok, :] = sum_k xt[:,k].T @ W[n][k]
                    o_ps = ps_o.tile([P, D], f32)
                    for k in range(KC):
                        nc.tensor.matmul(
                            out=o_ps[:],
                            lhsT=xt[:, k * P:(k + 1) * P],
                            rhs=w_sb[(n, k)][:],
                            start=(k == 0), stop=(k == KC - 1),
                        )
                    # 4) + bias (PSUM -> SBUF)
                    o_sb = opool.tile([P, D], f32, tag="osb")
                    nc.vector.tensor_add(
                        out=o_sb[:], in0=o_ps[:],
                        in1=bias_bc[:, n * D:(n + 1) * D],
                    )
                    # 5) scatter rows to original slots (pads -> trash rows)
                    nc.gpsimd.indirect_dma_start(
                        out=out_d[:, :],
                        out_offset=bass.IndirectOffsetOnAxis(
                            ap=sidx_sb[:, s0 + g:s0 + g + 1], axis=0),
                        in_=o_sb[:], in_offset=None,
                    )
    # Raw Bass skips Bacc's codegen_inst_isa_subclasses pass; without it the
    # library-reload InstISA has empty instr bytes ("ISA wrong length")
    lower_extended_insts(nc)
    return _split_multiwaits(nc)


def _routing(branch_idx, TS, NB):
    """Per-core padded, branch-sorted gather/scatter index arrays.

    Returns (gidx16 [NCORES][P, S*8] int16, sidx [NCORES][P, S] int32,
    branch_of_tile [S]). Gather pads read row 0; scatter pads write trash
    rows TS + partition."""
    ncores = branch_idx.shape[0] // TS
    perms, counts = [], np.zeros((ncores, NB), np.int64)
    for c in range(ncores):
        bi = branch_idx[c * TS:(c + 1) * TS]
        perms.append(np.argsort(bi, kind="stable"))
        counts[c] = np.bincount(bi, minlength=NB)
    slot_tiles = [int(-(-counts[:, n].max() // P)) for n in range(NB)]
    branch_of_tile = []
    for n in range(NB):
        branch_of_tile += [n] * slot_tiles[n]
    while len(branch_of_tile) % CH:      # pad to whole gather chunks
        branch_of_tile.append(NB - 1)
    S = len(branch_of_tile)

    gidx_arrays, sidx_arrays = [], []
    for c in range(ncores):
        flat = np.full(S * P, -1, np.int64)
        off = base = 0
        for n in range(NB):
            cnt = int(counts[c, n])
            flat[base:base + cnt] = perms[c][off:off + cnt]
            off += cnt
            base += slot_tiles[n] * P
        pad = flat < 0
        gflat = np.where(pad, 0, flat)
        sflat = np.where(pad, TS + (np.arange(S * P) % P), flat)
        # dma_gather index layout: slot i -> partition i%16, column i//16,
        # replicated across the 8 16-partition groups
        g16 = np.tile(gflat.reshape(-1, 16).T.astype(np.int16), (8, 1))
        gidx_arrays.append(np.ascontiguousarray(g16))
        sidx_arrays.append(
            np.ascontiguousarray(sflat.reshape(S, P).T.astype(np.int32)))
    return gidx_arrays, sidx_arrays, branch_of_tile


def kernel(x, branch_idx, weight, bias):
    from concourse.bass_utils import run_bass_kernel_spmd

    x = np.ascontiguousarray(np.asarray(x, np.float32))
    branch_idx = np.asarray(branch_idx, np.int32)
    weight = np.ascontiguousarray(np.asarray(weight, np.float32))
    bias = np.ascontiguousarray(np.asarray(bias, np.float32))

    T, D = x.shape
    NB = weight.shape[0]
    TS = T // NCORES

    gidx_arrays, sidx_arrays, branch_of_tile = _routing(branch_idx, TS, NB)

    key = (TS, D, NB, tuple(branch_of_tile))
    if key not in _prog_cache:
        _prog_cache[key] = _build_program(TS, D, NB, branch_of_tile)
    nc = _prog_cache[key]

    wr = np.ascontiguousarray(weight.reshape(NB * D, D))
    br = np.ascontiguousarray(bias.reshape(1, NB * D))
    in_maps = [
        {"x": x[c * TS:(c + 1) * TS], "wr": wr, "br": br,
         "gidx16": gidx_arrays[c], "sidx": sidx_arrays[c]}
        for c in range(NCORES)
    ]
    res = run_bass_kernel_spmd(nc, in_maps, core_ids=list(range(NCORES)))
    out = np.concatenate(
        [res.results[c]["out"][:TS] for c in range(NCORES)], axis=0)
    return out


# revision 16
# speedup vs baseline: 1.4309x; 1.2076x over previous
"""BranchLinear (MoE routing) Trainium2 kernel.

Math: out[t] = x[t] @ weight[branch_idx[t]] + bias[branch_idx[t]]
  x: [131072, 512] f32, branch_idx: [131072] int32 in [0,8),
  weight: [8, 512, 512] f32, bias: [8, 512] f32.

Strategy (data-parallel over 8 NeuronCores, T sharded):
  Per core (16384 tokens): tokens are processed grouped by branch so each
  token is multiplied by exactly one 512x512 weight (1x FLOPs, vs 8x for
  the masked approach). The grouping permutation is cheap host-side
  bookkeeping (argsort of the given routing); all FLOPs and all HBM
  traffic (x gather, weight load, out scatter) happen on device:
    - an indirect DMA gathers 128 sorted token rows per tile into SBUF
      (one 2KB descriptor per SBUF partition — the HW consumes exactly
      one index per partition, so multi-tile batching is not possible)
    - PE transposes the tile (via identity matmul) so D is on
      partitions; all matmul data is typed float32r, which streams at
      1 cycle/row on the PE (plain fp32 runs at 1/4 rate)
    - 4 accumulated matmuls against the resident branch weight
    - DVE adds the (pre-broadcast) branch bias
    - an indirect DMA scatters the 128 result rows to their original slots
  Branch segments are padded to 128-token tiles (pad slots gather row 0
  and scatter into trash rows TS..TS+127), and per-branch slot sizes are
  the max over cores so one SPMD program serves all 8 cores.
"""

import numpy as np

P = 128           # SBUF partitions / tile height (tokens per tile)
CH = 8            # tiles per dma_gather chunk (gmode="dma_gather" only)
GMODE = "indirect"   # "indirect" (per-tile InstDMACopy) | "dma_gather"
NCORES = 8

_prog_cache = {}


def _split_multiwaits(nc):
    """This container's walrus build allows at most ONE sync wait per
    instruction (2 for EventSemaphore), but Tile emits instructions with
    several waits. Hoist extra waits onto fresh single-wait nops inserted
    just before the instruction on the same engine (identical blocking
    semantics: the engine's sequencer executes both in program order)."""
    import concourse.mybir as mybir

    uid = 0
    for f in nc.m.functions:
        for bb in f.blocks:
            insts = bb.instructions
            out, changed = [], False
            for ins in insts:
                si = ins.sync_info
                cap = 2 if ins.opcode == "EventSemaphore" else 1
                if si is not None and len(si.on_wait) > cap:
                    waits = list(si.on_wait)
                    for w in waits[cap:]:
                        nop = mybir.InstNoOp(
                            name=f"waitsplit_{uid}",
                            engine=ins.engine,
                            bass_nofuse=True,
                            sync_info=mybir.SyncInfo(on_wait=[w], on_update=[]),
                        )
                        uid += 1
                        nc.register_instruction(nop, overwrite=True)
                        out.append(nop)
                    si.on_wait = waits[:cap]
                    ins.sync_info = si
                    changed = True
                out.append(ins)
            if changed:
                bb.instructions = out
    return nc


def _build_program(TS, D, NB, branch_of_tile, epochs=1, gmode=GMODE):
    """Build the per-core SPMD bass program.

    Inputs (per core): x [TS, D] f32r, wr [NB*D, D] f32r (weight
    reshaped), br [1, NB*D] f32, sidx [P, S] int32 (scatter pads write
    trash rows TS..TS+127), and the gather indices — gidx [P, S] int32
    (gmode="indirect"; pads read row 0) or gidx16 [P, S*8] int16
    (gmode="dma_gather": slot i at partition i%16, column i//16,
    replicated over the 8 16-partition groups). Output: out [TS+P, D]
    f32 (last P rows are trash). f32r is bit-identical to f32; it only
    switches the PE streaming mode.
    """
    import concourse.bass as bass
    import concourse.mybir as mybir
    import concourse.tile as tile
    from concourse import library_config
    from concourse.library_overlay import lower_extended_insts
    from concourse.masks import make_identity

    f32 = mybir.dt.float32
    f32r = mybir.dt.float32r
    KC = D // P                       # contraction chunks (4)
    S = len(branch_of_tile)           # total 128-token tiles
    assert S % CH == 0

    nc = bass.Bass(name="branch_linear")
    x_d = nc.dram_tensor("x", [TS, D], f32r, kind="ExternalInput")
    w_d = nc.dram_tensor("wr", [NB * D, D], f32r, kind="ExternalInput")
    b_d = nc.dram_tensor("br", [1, NB * D], f32, kind="ExternalInput")
    if gmode == "dma_gather":
        gidx_d = nc.dram_tensor("gidx16", [P, S * 8], mybir.dt.int16,
                                kind="ExternalInput")
    else:
        gidx_d = nc.dram_tensor("gidx", [P, S], mybir.dt.int32,
                                kind="ExternalInput")
    sidx_d = nc.dram_tensor("sidx", [P, S], mybir.dt.int32, kind="ExternalInput")
    out_d = nc.dram_tensor("out", [TS + P, D], f32, kind="ExternalOutput")

    with tile.TileContext(nc) as tc:
        with (
            tc.tile_pool(name="const", bufs=1) as cpool,
            tc.tile_pool(name="gather", bufs=2 if gmode == "dma_gather" else 6) as gpool,
            tc.tile_pool(name="xt", bufs=4) as tpool,
            tc.tile_pool(name="osb", bufs=4) as opool,
            tc.tile_pool(name="ps_t", bufs=2, space="PSUM") as ps_t,
            tc.tile_pool(name="ps_o", bufs=2, space="PSUM") as ps_o,
            tc.tile_pool(name="ps_b", bufs=1, space="PSUM") as ps_b,
        ):
            # memset rejects f32r, so build the identity in f32 and convert
            # with a (bit-identical) copy
            ident_f32 = cpool.tile([P, P], f32, tag="ident_f32")
            make_identity(nc, ident_f32[:])
            ident = cpool.tile([P, P], f32r, tag="ident")
            nc.vector.tensor_copy(out=ident[:], in_=ident_f32[:])

            if gmode == "dma_gather":
                # dma_gather lives in the mlp GPSIMD library; load it after
                # the make_identity gpsimd ops (Pool order is preserved).
                nc.gpsimd.load_library(library_config.mlp)
                # one shared constant register: to_reg inside every call
                # would exhaust the Pool register file on epochs>1 builds
                nidx_reg = nc.gpsimd.to_reg(CH * P)
                gidx_sb = cpool.tile([P, S * 8], mybir.dt.int16, tag="gidx16")
            else:
                gidx_sb = cpool.tile([P, S], mybir.dt.int32, tag="gidx")
            nc.sync.dma_start(gidx_sb[:], gidx_d[:, :])
            sidx_sb = cpool.tile([P, S], mybir.dt.int32, tag="sidx")
            nc.sync.dma_start(sidx_sb[:], sidx_d[:, :])

            # resident weights: one [P, D] SBUF tile per (branch, k-chunk)
            w_sb = {}
            for n in range(NB):
                for k in range(KC):
                    w = cpool.tile([P, D], f32r, tag=f"w_{n}_{k}")
                    r0 = (n * KC + k) * P
                    nc.sync.dma_start(w[:], w_d[r0:r0 + P, :])
                    w_sb[(n, k)] = w

            # bias, broadcast to 128 partitions via K=1 matmul with ones
            bias1p = cpool.tile([1, NB * D], f32, tag="bias1p")
            nc.sync.dma_start(bias1p[:], b_d[:, :])
            ones1p = cpool.tile([1, P], f32, tag="ones1p")
            nc.vector.memset(ones1p[:], 1.0)
            bias_bc = cpool.tile([P, NB * D], f32, tag="bias_bc")
            for n in range(NB):
                pb = ps_b.tile([P, D], f32)
                nc.tensor.matmul(
                    out=pb[:], lhsT=ones1p[:], rhs=bias1p[:, n * D:(n + 1) * D],
                    start=True, stop=True,
                )
                nc.scalar.copy(out=bias_bc[:, n * D:(n + 1) * D], in_=pb[:])

            def compute_tile(s, xg_slice):
                n = branch_of_tile[s]
                # 2) transpose tile so D is on partitions (PE, identity)
                xt_ps = ps_t.tile([P, D], f32r)
                for k in range(KC):
                    nc.tensor.transpose(
                        out=xt_ps[:, k * P:(k + 1) * P],
                        in_=xg_slice[:, k * P:(k + 1) * P],
                        identity=ident[:],
                    )
                xt = tpool.tile([P, D], f32r, tag="xt")
                nc.scalar.copy(out=xt[:], in_=xt_ps[:])
                # 3) out[tok, :] = sum_k xt[:,k].T @ W[n][k]
                o_ps = ps_o.tile([P, D], f32)
                for k in range(KC):
                    nc.tensor.matmul(
                        out=o_ps[:],
                        lhsT=xt[:, k * P:(k + 1) * P],
                        rhs=w_sb[(n, k)][:],
                        start=(k == 0), stop=(k == KC - 1),
                    )
                # 4) + bias (PSUM -> SBUF)
                o_sb = opool.tile([P, D], f32, tag="osb")
                nc.vector.tensor_add(
                    out=o_sb[:], in0=o_ps[:],
                    in1=bias_bc[:, n * D:(n + 1) * D],
                )
                # 5) scatter rows to original slots (pads -> trash rows)
                nc.gpsimd.indirect_dma_start(
                    out=out_d[:, :],
                    out_offset=bass.IndirectOffsetOnAxis(
                        ap=sidx_sb[:, s:s + 1], axis=0),
                    in_=o_sb[:], in_offset=None,
                )

            if gmode == "dma_gather":
                for s0 in list(range(0, S, CH)) * epochs:
                    # 1) batched gather: slot i -> partition i%128, tile
                    #    i//128 (pads read row 0)
                    xg = gpool.tile([P, CH, D], f32r, tag="xg")
                    nc.gpsimd.dma_gather(
                        xg[:], x_d[:, :],
                        gidx_sb[:, s0 * 8:(s0 + CH) * 8],
                        CH * P, nidx_reg, D,
                        single_packet=False,
                    )
                    for g in range(CH):
                        compute_tile(s0 + g, xg[:, g, :])
            else:
                for s in list(range(S)) * epochs:
                    # 1) gather 128 sorted token rows (pads read row 0)
                    xg = gpool.tile([P, D], f32r, tag="xg")
                    nc.gpsimd.indirect_dma_start(
                        out=xg[:], out_offset=None,
                        in_=x_d[:, :],
                        in_offset=bass.IndirectOffsetOnAxis(
                            ap=gidx_sb[:, s:s + 1], axis=0),
                    )
                    compute_tile(s, xg[:])
    # Raw Bass skips Bacc's codegen_inst_isa_subclasses pass; without it the
    # library-reload InstISA has empty instr bytes ("ISA wrong length")
    lower_extended_insts(nc)
    return _split_multiwaits(nc)


def _routing(branch_idx, TS, NB):
    """Per-core padded, branch-sorted gather/scatter index arrays.

    Returns (gidx [NCORES][P, S] int32, gidx16 [NCORES][P, S*8] int16,
    sidx [NCORES][P, S] int32, branch_of_tile [S]). Gather pads read
    row 0; scatter pads write trash rows TS + partition."""
    ncores = branch_idx.shape[0] // TS
    perms, counts = [], np.zeros((ncores, NB), np.int64)
    for c in range(ncores):
        bi = branch_idx[c * TS:(c + 1) * TS]
        perms.append(np.argsort(bi, kind="stable"))
        counts[c] = np.bincount(bi, minlength=NB)
    slot_tiles = [int(-(-counts[:, n].max() // P)) for n in range(NB)]
    branch_of_tile = []
    for n in range(NB):
        branch_of_tile += [n] * slot_tiles[n]
    while len(branch_of_tile) % CH:      # pad to whole gather chunks
        branch_of_tile.append(NB - 1)
    S = len(branch_of_tile)

    gidx_arrays, g16_arrays, sidx_arrays = [], [], []
    for c in range(ncores):
        flat = np.full(S * P, -1, np.int64)
        off = base = 0
        for n in range(NB):
            cnt = int(counts[c, n])
            flat[base:base + cnt] = perms[c][off:off + cnt]
            off += cnt
            base += slot_tiles[n] * P
        pad = flat < 0
        gflat = np.where(pad, 0, flat)
        sflat = np.where(pad, TS + (np.arange(S * P) % P), flat)
        gidx_arrays.append(
            np.ascontiguousarray(gflat.reshape(S, P).T.astype(np.int32)))
        # dma_gather index layout: slot i -> partition i%16, column i//16,
        # replicated across the 8 16-partition groups
        g16 = np.tile(gflat.reshape(-1, 16).T.astype(np.int16), (8, 1))
        g16_arrays.append(np.ascontiguousarray(g16))
        sidx_arrays.append(
            np.ascontiguousarray(sflat.reshape(S, P).T.astype(np.int32)))
    return gidx_arrays, g16_arrays, sidx_arrays, branch_of_tile


def make_in_maps(x, wr, br, gidx, g16, sidx, TS, gmode=GMODE):
    maps = []
    for c in range(NCORES):
        m = {"x": x[c * TS:(c + 1) * TS], "wr": wr, "br": br,
             "sidx": sidx[c]}
        if gmode == "dma_gather":
            m["gidx16"] = g16[c]
        else:
            m["gidx"] = gidx[c]
        maps.append(m)
    return maps


def kernel(x, branch_idx, weight, bias):
    from concourse.bass_utils import run_bass_kernel_spmd

    x = np.ascontiguousarray(np.asarray(x, np.float32))
    branch_idx = np.asarray(branch_idx, np.int32)
    weight = np.ascontiguousarray(np.asarray(weight, np.float32))
    bias = np.ascontiguousarray(np.asarray(bias, np.float32))

    T, D = x.shape
    NB = weight.shape[0]
    TS = T // NCORES

    gidx, g16, sidx, branch_of_tile = _routing(branch_idx, TS, NB)

    key = (TS, D, NB, tuple(branch_of_tile), GMODE)
    if key not in _prog_cache:
        _prog_cache[key] = _build_program(TS, D, NB, branch_of_tile)
    nc = _prog_cache[key]

    wr = np.ascontiguousarray(weight.reshape(NB * D, D))
    br = np.ascontiguousarray(bias.reshape(1, NB * D))
    in_maps = make_in_maps(x, wr, br, gidx, g16, sidx, TS)
    res = run_bass_kernel_spmd(nc, in_maps, core_ids=list(range(NCORES)))
    out = np.concatenate(
        [res.results[c]["out"][:TS] for c in range(NCORES)], axis=0)
    return out
